# revision 1
# baseline (speedup 1.0000x reference)
"""RBF kernel matrix on 8 TRN2 NeuronCores.

Computes out[i, j] = exp(-gamma * max(||x_i||^2 + ||y_j||^2 - 2 x_i.y_j, 0))
with gamma = softplus(MLP(x[0])) + 1e-6, as a Bass/Tile SPMD kernel.

Sharding: rows of x across the 8 cores (1024 rows each); y and the tiny
gamma-net parameters are replicated.  Each core computes its (1024, 8192)
slab of the output; the host concatenates the slabs.

Per-core device pipeline:
  - gamma chain (TensorE f32 matmuls + ACT relu / exp / ln -> softplus)
  - fold -2*gamma into x^T (DVE), square x^T/y^T (DVE), reduce the squares
    over d with ones-matmuls scaled by -gamma (TensorE) -> row vectors
    A = [-g*||x||^2 ; 1] (2,1024) and B = [1 ; -g*||y||^2] (2,8192)
  - main loop: psum[1024-col group] = (-2g x^T).T @ y^T (2 bf16 matmuls)
    + A.T @ B (K=2 matmul) == -gamma * dist^2; one ACT Exp per group
    PSUM->SBUF; 2 MB DMAs to DRAM.
"""

import numpy as np
import ml_dtypes

import concourse.bacc as bacc
import concourse.bass as bass  # noqa: F401
import concourse.mybir as mybir
import concourse.tile as tile
from concourse.bass_utils import run_bass_kernel_spmd

N_CORES = 8
N, M, D = 8192, 8192, 256
N_SH = N // N_CORES  # rows of x per core
HID = 16
P = 128
KC = D // P  # k-chunks (2)

F32 = mybir.dt.float32
BF16 = mybir.dt.bfloat16
AF = mybir.ActivationFunctionType
ALU = mybir.AluOpType

_NC = None
LAST_RESULT = None


def _ensure_ntff_hook():
    """Register an ``antenv.axon_hooks`` shim if the image lacks it.

    ``run_bass_kernel_spmd(trace=True)`` under axon imports
    ``antenv.axon_hooks.get_axon_ntff_profile_hook``; some images miss the
    module, which would crash tracing.  Recreate the boot-script hook via
    ctypes against libaxon_pjrt.so, degrading to hook=None when absent.
    """
    import contextlib
    import ctypes
    import os
    import sys
    import types

    try:
        import antenv.axon_hooks  # noqa: F401
        return
    except ImportError:
        pass

    hook = None
    so_path = "/opt/axon/libaxon_pjrt.so"
    if os.path.exists(so_path):
        try:
            lib = ctypes.CDLL(so_path)
            if hasattr(lib, "axon_start_nrt_profile"):
                lib.axon_start_nrt_profile.argtypes = [
                    ctypes.POINTER(ctypes.c_int64), ctypes.c_size_t]
                lib.axon_start_nrt_profile.restype = ctypes.c_int64
                lib.axon_stop_nrt_profile.argtypes = [ctypes.c_char_p]
                lib.axon_stop_nrt_profile.restype = ctypes.c_int64

                @contextlib.contextmanager
                def _hook(output_dir, device_ids):
                    import jax
                    jax.devices()
                    if device_ids:
                        ids = (ctypes.c_int64 * len(device_ids))(*device_ids)
                        rc = lib.axon_start_nrt_profile(ids, len(device_ids))
                    else:
                        rc = lib.axon_start_nrt_profile(None, 0)
                    if rc != 0:
                        raise RuntimeError(f"axon_start_nrt_profile rc={rc}")
                    try:
                        yield
                    finally:
                        n = lib.axon_stop_nrt_profile(str(output_dir).encode())
                        if n <= 0:
                            print(f"ntff profile capture wrote {n} files",
                                  file=sys.stderr)

                hook = _hook
        except OSError:
            hook = None

    mod = types.ModuleType("antenv.axon_hooks")
    mod._hook = hook
    mod.get_axon_ntff_profile_hook = lambda: mod._hook

    def _set(h):
        mod._hook = h

    mod.set_axon_ntff_profile_hook = _set
    sys.modules["antenv.axon_hooks"] = mod
    try:
        import antenv
        antenv.axon_hooks = mod
    except ImportError:
        pass


_ensure_ntff_hook()


def _build_nc():
    nc = bacc.Bacc("TRN2", target_bir_lowering=False, debug=False,
                   num_devices=N_CORES)

    xt_d = nc.dram_tensor("xt", [KC, P, N_SH], BF16, kind="ExternalInput")
    yt_d = nc.dram_tensor("yt", [KC, P, M], BF16, kind="ExternalInput")
    x0_d = nc.dram_tensor("x0", [KC, P, 1], F32, kind="ExternalInput")
    w1t_d = nc.dram_tensor("w1t", [KC, P, HID], F32, kind="ExternalInput")
    b1_d = nc.dram_tensor("b1", [HID, 1], F32, kind="ExternalInput")
    w2t_d = nc.dram_tensor("w2t", [HID, 1], F32, kind="ExternalInput")
    b2_d = nc.dram_tensor("b2", [1, 1], F32, kind="ExternalInput")
    out_d = nc.dram_tensor("out", [N_SH, M], F32, kind="ExternalOutput")

    with tile.TileContext(nc) as tc:
        with (
            tc.tile_pool(name="const", bufs=1) as const,
            tc.tile_pool(name="work", bufs=3) as work,
            tc.tile_pool(name="stage", bufs=2) as stage_pool,
            tc.tile_pool(name="psmm", bufs=3, space="PSUM") as psmm,
            tc.tile_pool(name="psprep", bufs=2, space="PSUM") as psprep,
        ):
            # ---------------- gamma chain ----------------
            x0_sb = const.tile([P, KC, 1], F32)
            w1t_sb = const.tile([P, KC, HID], F32)
            b1_sb = const.tile([HID, 1], F32)
            w2t_sb = const.tile([HID, 1], F32)
            b2_sb = const.tile([1, 1], F32)
            for k in range(KC):
                nc.sync.dma_start(x0_sb[:, k], x0_d[k])
                nc.sync.dma_start(w1t_sb[:, k], w1t_d[k])
            nc.sync.dma_start(b1_sb[:], b1_d[:])
            nc.sync.dma_start(w2t_sb[:], w2t_d[:])
            nc.sync.dma_start(b2_sb[:], b2_d[:])

            # Funnel the gamma-chain matmul inputs through DVE copies: fp32
            # self-loading matmuls encode only ONE sync wait, but these DMAs
            # land on several DMA semaphore lanes.  After the copies every
            # gamma matmul waits on the single DVE semaphore.
            x0_c = const.tile([P, KC, 1], F32)
            w1t_c = const.tile([P, KC, HID], F32)
            w2t_c = const.tile([HID, 1], F32)
            nc.vector.tensor_copy(x0_c[:], x0_sb[:])
            nc.vector.tensor_copy(w1t_c[:], w1t_sb[:])
            nc.vector.tensor_copy(w2t_c[:], w2t_sb[:])

            ps_h = psprep.tile([HID, 1], F32, tag="prep")
            for k in range(KC):
                nc.tensor.matmul(ps_h[:], w1t_c[:, k], x0_c[:, k],
                                 start=(k == 0), stop=(k == KC - 1))
            h_sb = const.tile([HID, 1], F32)
            nc.scalar.activation(h_sb[:], ps_h[:], AF.Relu, bias=b1_sb[:])

            ps_z = psprep.tile([1, 1], F32, tag="prep")
            nc.tensor.matmul(ps_z[:], w2t_c[:], h_sb[:], start=True, stop=True)
            u_sb = const.tile([1, 1], F32)
            nc.scalar.activation(u_sb[:], ps_z[:], AF.Exp, bias=b2_sb[:])
            s_sb = const.tile([1, 1], F32)  # softplus(z) = ln(1 + e^z)
            nc.scalar.activation(s_sb[:], u_sb[:], AF.Ln, bias=1.0)

            ones_row = const.tile([1, P], F32)
            nc.vector.memset(ones_row[:], 1.0)
            ps_g = psprep.tile([P, 1], F32, tag="prep")
            nc.tensor.matmul(ps_g[:], ones_row[:], s_sb[:], start=True, stop=True)

            negg_f = const.tile([P, 1], F32)     # -gamma on every partition
            nc.vector.tensor_scalar(negg_f[:], ps_g[:], -1.0, -1e-6,
                                    ALU.mult, ALU.add)
            neg2g_f = const.tile([P, 1], F32)    # -2*gamma
            nc.vector.tensor_scalar(neg2g_f[:], ps_g[:], -2.0, -2e-6,
                                    ALU.mult, ALU.add)
            # Stationary operands used to build the K=2 "norms" rows fully
            # inside PSUM (engine writes must start at partition 0, so the
            # constant `ones` row cannot be memset at partition 1 directly).
            # L_negA: col1 = -gamma -> psum row1 = -g*||x||^2; L_oneA adds 1
            # to row0.  L_negB/L_oneB mirror this for B (data in row0).
            l_negA = const.tile([P, 2], BF16)
            nc.vector.memset(l_negA[:], 0.0)
            nc.vector.tensor_copy(l_negA[:, 1:2], negg_f[:])
            l_negB = const.tile([P, 2], BF16)
            nc.vector.memset(l_negB[:], 0.0)
            nc.vector.tensor_copy(l_negB[:, 0:1], negg_f[:])
            l_oneA = const.tile([1, 2], BF16)
            nc.vector.memset(l_oneA[:], 0.0)
            nc.vector.memset(l_oneA[:, 0:1], 1.0)
            l_oneB = const.tile([1, 2], BF16)
            nc.vector.memset(l_oneB[:], 0.0)
            nc.vector.memset(l_oneB[:, 1:2], 1.0)
            ones512 = const.tile([1, 512], BF16)
            nc.vector.memset(ones512[:], 1.0)

            # ---------------- x side ----------------
            xT_sb = const.tile([P, KC, N_SH], BF16)
            for k in range(KC):
                nc.sync.dma_start(xT_sb[:, k], xt_d[k])

            xs_sb = const.tile([P, KC, N_SH], BF16)  # (-2 gamma) * x^T
            sqx = work.tile([P, KC, N_SH], BF16, tag="sqx")
            for k in range(KC):
                nc.vector.tensor_scalar(xs_sb[:, k], xT_sb[:, k], neg2g_f[:],
                                        None, ALU.mult)
                nc.vector.tensor_tensor(sqx[:, k], xT_sb[:, k], xT_sb[:, k],
                                        ALU.mult)

            # A = [ones ; -g*||x||^2]  (2, N_SH)
            A_sb = const.tile([2, N_SH], BF16)
            for g in range(N_SH // 512):
                sl = slice(g * 512, (g + 1) * 512)
                ps_r = psprep.tile([2, 512], F32, tag="prep")
                for k in range(KC):
                    nc.tensor.matmul(ps_r[:], l_negA[:], sqx[:, k, sl],
                                     start=(k == 0), stop=False)
                nc.tensor.matmul(ps_r[:], l_oneA[:], ones512[:],
                                 start=False, stop=True)
                nc.vector.tensor_copy(A_sb[:, sl], ps_r[:])

            # ---------------- y side ----------------
            # B = [-g*||y||^2 ; ones]  (2, M)
            yT_sb = const.tile([P, KC, M], BF16)
            B_sb = const.tile([2, M], BF16)
            for c in range(M // 512):
                sl = slice(c * 512, (c + 1) * 512)
                for k in range(KC):
                    nc.sync.dma_start(yT_sb[:, k, sl], yt_d[k, :, sl])
                sqy = work.tile([P, KC, 512], BF16, tag="sqy")
                for k in range(KC):
                    nc.vector.tensor_tensor(sqy[:, k], yT_sb[:, k, sl],
                                            yT_sb[:, k, sl], ALU.mult)
                ps_r = psprep.tile([2, 512], F32, tag="prep")
                for k in range(KC):
                    nc.tensor.matmul(ps_r[:], l_negB[:], sqy[:, k],
                                     start=(k == 0), stop=False)
                nc.tensor.matmul(ps_r[:], l_oneB[:], ones512[:],
                                 start=False, stop=True)
                nc.vector.tensor_copy(B_sb[:, sl], ps_r[:])

            # ---------------- main loop ----------------
            GCOL = 1024           # psum group columns (2 banks)
            SCOL = 4096           # staging columns per DMA (2 MB)
            for m in range(N_SH // P):
                msl = slice(m * P, (m + 1) * P)
                for half in range(M // SCOL):
                    stage = stage_pool.tile([P, SCOL], F32, tag="out")
                    for gg in range(SCOL // GCOL):
                        col0 = half * SCOL + gg * GCOL
                        ps = psmm.tile([P, GCOL], F32, tag="mm")
                        for k in range(KC):
                            lhsT = xs_sb[:, k, msl]
                            for j in range(GCOL // 512):
                                nc.tensor.matmul(
                                    ps[:, j * 512:(j + 1) * 512], lhsT,
                                    yT_sb[:, k, col0 + j * 512:col0 + (j + 1) * 512],
                                    start=(k == 0), stop=False)
                        for j in range(GCOL // 512):
                            nc.tensor.matmul(
                                ps[:, j * 512:(j + 1) * 512], A_sb[:, msl],
                                B_sb[:, col0 + j * 512:col0 + (j + 1) * 512],
                                start=False, stop=True)
                        nc.scalar.activation(
                            stage[:, gg * GCOL:(gg + 1) * GCOL], ps[:], AF.Exp)
                    nc.sync.dma_start(
                        out_d[msl, half * SCOL:(half + 1) * SCOL], stage[:])
    nc.compile()
    return nc


def _get_nc():
    global _NC
    if _NC is None:
        _NC = _build_nc()
    return _NC


def kernel(x, y, W1, b1, W2, b2):
    global LAST_RESULT
    x = np.asarray(x, dtype=np.float32)
    y = np.asarray(y, dtype=np.float32)
    bf = ml_dtypes.bfloat16

    yt = np.ascontiguousarray(y.T).reshape(KC, P, M).astype(bf)
    x0 = np.ascontiguousarray(x[0]).reshape(KC, P, 1).astype(np.float32)
    w1t = np.ascontiguousarray(np.asarray(W1, np.float32).T).reshape(KC, P, HID)
    b1c = np.asarray(b1, np.float32).reshape(HID, 1)
    w2t = np.ascontiguousarray(np.asarray(W2, np.float32).T).reshape(HID, 1)
    b2c = np.asarray(b2, np.float32).reshape(1, 1)

    in_maps = []
    for c in range(N_CORES):
        shard = x[c * N_SH:(c + 1) * N_SH]
        xt = np.ascontiguousarray(shard.T).reshape(KC, P, N_SH).astype(bf)
        in_maps.append({"xt": xt, "yt": yt, "x0": x0, "w1t": w1t,
                        "b1": b1c, "w2t": w2t, "b2": b2c})

    nc = _get_nc()
    LAST_RESULT = run_bass_kernel_spmd(nc, in_maps, core_ids=list(range(N_CORES)))
    return np.concatenate([LAST_RESULT.results[c]["out"]
                           for c in range(N_CORES)], axis=0)



# revision 2
# speedup vs baseline: 1.5382x; 1.5382x over previous
"""RBF kernel matrix on 8 TRN2 NeuronCores.

Computes out[i, j] = exp(-gamma * (||x_i||^2 + ||y_j||^2 - 2 x_i.y_j))
with gamma = softplus(MLP(x[0])) + 1e-6, as a Bass/Tile SPMD kernel.

Sharding: rows of x across the 8 cores (1024 rows each); y and the tiny
gamma-net parameters are replicated.  Each core computes its (1024, 8192)
slab of the output; the host concatenates the slabs.

Per-core device pipeline (keeps the PE free of norm-handling matmuls):
  - gamma chain (TensorE f32 matmuls + ACT relu / exp / ln -> softplus)
  - Bb[p, j] = -0.5*||y_j||^2 broadcast to all 128 partitions, built with
    K=128 matmuls against a constant (-0.5) stationary matrix over sqy
  - x-row norms from native x via DVE square+accum -> per-partition bias
  - main loop per [128, 1024] tile: psum = x.y (2 bf16 K=128 matmuls),
    DVE psum += Bb slice, ACT stage = exp(2g*psum - g*||x_i||^2) using the
    per-partition scale/bias operands; 2 MB DMAs to DRAM.
"""

import numpy as np
import ml_dtypes

import concourse.bacc as bacc
import concourse.bass as bass  # noqa: F401
import concourse.mybir as mybir
import concourse.tile as tile
from concourse.bass_utils import run_bass_kernel_spmd

N_CORES = 8
N, M, D = 8192, 8192, 256
N_SH = N // N_CORES  # rows of x per core
HID = 16
P = 128
KC = D // P  # k-chunks (2)
MT = N_SH // P  # m-tiles per core (8)

F32 = mybir.dt.float32
BF16 = mybir.dt.bfloat16
AF = mybir.ActivationFunctionType
ALU = mybir.AluOpType

_NC = None
LAST_RESULT = None


def _ensure_ntff_hook():
    """Register an ``antenv.axon_hooks`` shim if the image lacks it.

    ``run_bass_kernel_spmd(trace=True)`` under axon imports
    ``antenv.axon_hooks.get_axon_ntff_profile_hook``; some images miss the
    module, which would crash tracing.  Recreate the boot-script hook via
    ctypes against libaxon_pjrt.so, degrading to hook=None when absent.
    """
    import contextlib
    import ctypes
    import os
    import sys
    import types

    try:
        import antenv.axon_hooks  # noqa: F401
        return
    except ImportError:
        pass

    hook = None
    so_path = "/opt/axon/libaxon_pjrt.so"
    if os.path.exists(so_path):
        try:
            lib = ctypes.CDLL(so_path)
            if hasattr(lib, "axon_start_nrt_profile"):
                lib.axon_start_nrt_profile.argtypes = [
                    ctypes.POINTER(ctypes.c_int64), ctypes.c_size_t]
                lib.axon_start_nrt_profile.restype = ctypes.c_int64
                lib.axon_stop_nrt_profile.argtypes = [ctypes.c_char_p]
                lib.axon_stop_nrt_profile.restype = ctypes.c_int64

                @contextlib.contextmanager
                def _hook(output_dir, device_ids):
                    import jax
                    jax.devices()
                    if device_ids:
                        ids = (ctypes.c_int64 * len(device_ids))(*device_ids)
                        rc = lib.axon_start_nrt_profile(ids, len(device_ids))
                    else:
                        rc = lib.axon_start_nrt_profile(None, 0)
                    if rc != 0:
                        raise RuntimeError(f"axon_start_nrt_profile rc={rc}")
                    try:
                        yield
                    finally:
                        n = lib.axon_stop_nrt_profile(str(output_dir).encode())
                        if n <= 0:
                            print(f"ntff profile capture wrote {n} files",
                                  file=sys.stderr)

                hook = _hook
        except OSError:
            hook = None

    mod = types.ModuleType("antenv.axon_hooks")
    mod._hook = hook
    mod.get_axon_ntff_profile_hook = lambda: mod._hook

    def _set(h):
        mod._hook = h

    mod.set_axon_ntff_profile_hook = _set
    sys.modules["antenv.axon_hooks"] = mod
    try:
        import antenv
        antenv.axon_hooks = mod
    except ImportError:
        pass


_ensure_ntff_hook()


def _build_nc():
    nc = bacc.Bacc("TRN2", target_bir_lowering=False, debug=False,
                   num_devices=N_CORES)

    xt_d = nc.dram_tensor("xt", [KC, P, N_SH], BF16, kind="ExternalInput")
    xr_d = nc.dram_tensor("xr", [P, MT, D], F32, kind="ExternalInput")
    yt_d = nc.dram_tensor("yt", [P, KC, M], BF16, kind="ExternalInput")
    x0_d = nc.dram_tensor("x0", [KC, P, 1], F32, kind="ExternalInput")
    w1t_d = nc.dram_tensor("w1t", [KC, P, HID], F32, kind="ExternalInput")
    b1_d = nc.dram_tensor("b1", [HID, 1], F32, kind="ExternalInput")
    w2t_d = nc.dram_tensor("w2t", [HID, 1], F32, kind="ExternalInput")
    b2_d = nc.dram_tensor("b2", [1, 1], F32, kind="ExternalInput")
    out_d = nc.dram_tensor("out", [N_SH, M], F32, kind="ExternalOutput")

    YP = 1024  # y columns per input-DMA piece
    GCOL = 1024  # main-loop psum group columns (2 banks)
    SCOL = 4096  # staging columns per output DMA (2 MB)

    with tile.TileContext(nc) as tc:
        with (
            tc.tile_pool(name="const", bufs=1) as const,
            tc.tile_pool(name="work", bufs=3) as work,
            tc.tile_pool(name="stage", bufs=3) as stage_pool,
            tc.tile_pool(name="ps", bufs=1, space="PSUM") as ps_pool,
        ):
            # ---------------- input DMAs ----------------
            xT_sb = const.tile([P, KC, N_SH], BF16)
            for k in range(KC):
                nc.sync.dma_start(xT_sb[:, k], xt_d[k])
            xr_sb = const.tile([P, MT, D], F32)
            nc.sync.dma_start(xr_sb[:], xr_d[:])
            yT_sb = const.tile([P, KC, M], BF16)
            for c in range(M // YP):
                sl = slice(c * YP, (c + 1) * YP)
                nc.sync.dma_start(yT_sb[:, :, sl], yt_d[:, :, sl])

            x0_sb = const.tile([P, KC, 1], F32)
            w1t_sb = const.tile([P, KC, HID], F32)
            b1_sb = const.tile([HID, 1], F32)
            w2t_sb = const.tile([HID, 1], F32)
            b2_sb = const.tile([1, 1], F32)
            for k in range(KC):
                nc.sync.dma_start(x0_sb[:, k], x0_d[k])
                nc.sync.dma_start(w1t_sb[:, k], w1t_d[k])
            nc.sync.dma_start(b1_sb[:], b1_d[:])
            nc.sync.dma_start(w2t_sb[:], w2t_d[:])
            nc.sync.dma_start(b2_sb[:], b2_d[:])

            # ---------------- gamma chain ----------------
            # Funnel the gamma-chain matmul inputs through DVE copies: fp32
            # self-loading matmuls encode only ONE sync wait, but these DMAs
            # land on several DMA semaphore lanes.  After the copies every
            # gamma matmul waits on the single DVE semaphore.
            x0_c = const.tile([P, KC, 1], F32)
            w1t_c = const.tile([P, KC, HID], F32)
            w2t_c = const.tile([HID, 1], F32)
            nc.vector.tensor_copy(x0_c[:], x0_sb[:])
            nc.vector.tensor_copy(w1t_c[:], w1t_sb[:])
            nc.vector.tensor_copy(w2t_c[:], w2t_sb[:])

            ps_h = ps_pool.tile([HID, 1], F32, tag="prep", bufs=2)
            for k in range(KC):
                nc.tensor.matmul(ps_h[:], w1t_c[:, k], x0_c[:, k],
                                 start=(k == 0), stop=(k == KC - 1))
            h_sb = const.tile([HID, 1], F32)
            nc.scalar.activation(h_sb[:], ps_h[:], AF.Relu, bias=b1_sb[:])

            ps_z = ps_pool.tile([1, 1], F32, tag="prep", bufs=2)
            nc.tensor.matmul(ps_z[:], w2t_c[:], h_sb[:], start=True, stop=True)
            u_sb = const.tile([1, 1], F32)
            nc.scalar.activation(u_sb[:], ps_z[:], AF.Exp, bias=b2_sb[:])
            s_sb = const.tile([1, 1], F32)  # softplus(z) = ln(1 + e^z)
            nc.scalar.activation(s_sb[:], u_sb[:], AF.Ln, bias=1.0)

            ones_row = const.tile([1, P], F32)
            nc.vector.memset(ones_row[:], 1.0)
            ps_g = ps_pool.tile([P, 1], F32, tag="prep", bufs=2)
            nc.tensor.matmul(ps_g[:], ones_row[:], s_sb[:], start=True, stop=True)

            negg_f = const.tile([P, 1], F32)     # -gamma on every partition
            nc.vector.tensor_scalar(negg_f[:], ps_g[:], -1.0, -1e-6,
                                    ALU.mult, ALU.add)
            p2g_f = const.tile([P, 1], F32)      # +2*gamma
            nc.vector.tensor_scalar(p2g_f[:], ps_g[:], 2.0, 2e-6,
                                    ALU.mult, ALU.add)

            # ---------------- Bb = -0.5*||y_j||^2 on all partitions -------
            negh = const.tile([P, P], BF16)
            nc.vector.memset(negh[:], -0.5)
            Bb = const.tile([P, M], F32)
            for g in range(M // 512):
                sl = slice(g * 512, (g + 1) * 512)
                sqy = work.tile([P, KC, 512], BF16, tag="sqy")
                nc.vector.tensor_tensor(sqy[:], yT_sb[:, :, sl],
                                        yT_sb[:, :, sl], ALU.mult)
                ps_b = ps_pool.tile([P, 512], F32, tag="prep", bufs=2)
                for k in range(KC):
                    nc.tensor.matmul(ps_b[:], negh[:], sqy[:, k],
                                     start=(k == 0), stop=(k == KC - 1))
                nc.scalar.copy(Bb[:, sl], ps_b[:])

            # ---------------- x-row norms -> per-partition bias -----------
            xn = const.tile([P, MT], F32)
            for m in range(MT):
                sq_scr = work.tile([P, D], F32, tag="sqx")
                nc.vector.scalar_tensor_tensor(
                    sq_scr[:], xr_sb[:, m], 1.0, xr_sb[:, m],
                    ALU.mult, ALU.mult, accum_out=xn[:, m:m + 1])
            negxn = const.tile([P, MT], F32)     # -gamma * ||x_i||^2
            nc.vector.tensor_scalar(negxn[:], xn[:], negg_f[:], None, ALU.mult)

            # ---------------- main loop ----------------
            for m in range(MT):
                msl = slice(m * P, (m + 1) * P)
                for half in range(M // SCOL):
                    stg = stage_pool.tile([P, SCOL], F32, tag="out")
                    for gg in range(SCOL // GCOL):
                        col0 = half * SCOL + gg * GCOL
                        ps = ps_pool.tile([P, GCOL], F32, tag="mm", bufs=3)
                        for k in range(KC):
                            lhsT = xT_sb[:, k, msl]
                            for j in range(GCOL // 512):
                                nc.tensor.matmul(
                                    ps[:, j * 512:(j + 1) * 512], lhsT,
                                    yT_sb[:, k, col0 + j * 512:col0 + (j + 1) * 512],
                                    start=(k == 0), stop=(k == KC - 1))
                        nc.vector.tensor_tensor(
                            ps[:], ps[:], Bb[:, col0:col0 + GCOL], ALU.add)
                        nc.scalar.activation(
                            stg[:, gg * GCOL:(gg + 1) * GCOL], ps[:], AF.Exp,
                            bias=negxn[:, m:m + 1], scale=p2g_f[:])
                    nc.sync.dma_start(
                        out_d[msl, half * SCOL:(half + 1) * SCOL], stg[:])
    nc.compile()
    return nc


def _get_nc():
    global _NC
    if _NC is None:
        _NC = _build_nc()
    return _NC


def kernel(x, y, W1, b1, W2, b2):
    global LAST_RESULT
    x = np.asarray(x, dtype=np.float32)
    y = np.asarray(y, dtype=np.float32)
    bf = ml_dtypes.bfloat16

    # y^T chunked [128, KC, M]: element [p, k, j] = y[j, k*128+p]
    yt = np.ascontiguousarray(
        y.T.reshape(KC, P, M).transpose(1, 0, 2)).astype(bf)
    x0 = np.ascontiguousarray(x[0]).reshape(KC, P, 1).astype(np.float32)
    w1t = np.ascontiguousarray(np.asarray(W1, np.float32).T).reshape(KC, P, HID)
    b1c = np.asarray(b1, np.float32).reshape(HID, 1)
    w2t = np.ascontiguousarray(np.asarray(W2, np.float32).T).reshape(HID, 1)
    b2c = np.asarray(b2, np.float32).reshape(1, 1)

    in_maps = []
    for c in range(N_CORES):
        shard = x[c * N_SH:(c + 1) * N_SH]
        xt = np.ascontiguousarray(shard.T).reshape(KC, P, N_SH).astype(bf)
        xr = np.ascontiguousarray(
            shard.reshape(MT, P, D).transpose(1, 0, 2))
        in_maps.append({"xt": xt, "xr": xr, "yt": yt, "x0": x0, "w1t": w1t,
                        "b1": b1c, "w2t": w2t, "b2": b2c})

    nc = _get_nc()
    LAST_RESULT = run_bass_kernel_spmd(nc, in_maps, core_ids=list(range(N_CORES)))
    return np.concatenate([LAST_RESULT.results[c]["out"]
                           for c in range(N_CORES)], axis=0)


# revision 3
# speedup vs baseline: 1.5407x; 1.0016x over previous
"""RBF kernel matrix on 8 TRN2 NeuronCores.

Computes out[i, j] = exp(-gamma * (||x_i||^2 + ||y_j||^2 - 2 x_i.y_j))
with gamma = softplus(MLP(x[0])) + 1e-6, as a Bass/Tile SPMD kernel.

Sharding: rows of x across the 8 cores (1024 rows each); y and the tiny
gamma-net parameters are replicated.  Each core computes its (1024, 8192)
slab of the output; the host concatenates the slabs.

Per-core device pipeline (keeps the PE free of norm-handling matmuls):
  - gamma chain (TensorE f32 matmuls + ACT relu / exp / ln -> softplus)
  - Ey[j] = exp(-gamma*||y_j||^2) built once: DVE squares y^T, K=128
    matmuls against a constant (-0.5) stationary matrix reduce them, ACT
    exponentiates with the per-partition 2*gamma scale
  - x-row norms from native x via DVE square+accum -> per-partition bias
  - main loop per [128, 2048] tile: psum = x.y (bf16 K=128 matmuls), ACT
    stage = exp(2g*psum - g*||x_i||^2) in bf16, DVE stage *= Ey (bf16 2x
    mode), SWDGE output DMA casts bf16 -> f32 on the fly.
The multiplicative split exp(a+b) = exp(a)*exp(b) is safe here: both
factors are ~exp(-gamma*O(d)) << 1 for this input distribution, so
neither factor can overflow on its own.
"""

import numpy as np
import ml_dtypes

import concourse.bacc as bacc
import concourse.bass as bass  # noqa: F401
import concourse.mybir as mybir
import concourse.tile as tile
from concourse.bass_utils import run_bass_kernel_spmd

N_CORES = 8
N, M, D = 8192, 8192, 256
N_SH = N // N_CORES  # rows of x per core
HID = 16
P = 128
KC = D // P  # k-chunks (2)
MT = N_SH // P  # m-tiles per core (8)
YP = 1024  # y columns per input-DMA piece
NP = M // YP  # pieces (8)
PANEL = 2048  # main-loop columns per panel / psum tile
NPAN = M // PANEL  # panels (4)

F32 = mybir.dt.float32
BF16 = mybir.dt.bfloat16
AF = mybir.ActivationFunctionType
ALU = mybir.AluOpType

_NC = None
LAST_RESULT = None


def _ensure_ntff_hook():
    """Register an ``antenv.axon_hooks`` shim if the image lacks it.

    ``run_bass_kernel_spmd(trace=True)`` under axon imports
    ``antenv.axon_hooks.get_axon_ntff_profile_hook``; some images miss the
    module, which would crash tracing.  Recreate the boot-script hook via
    ctypes against libaxon_pjrt.so, degrading to hook=None when absent.
    """
    import contextlib
    import ctypes
    import os
    import sys
    import types

    try:
        import antenv.axon_hooks  # noqa: F401
        return
    except ImportError:
        pass

    hook = None
    so_path = "/opt/axon/libaxon_pjrt.so"
    if os.path.exists(so_path):
        try:
            lib = ctypes.CDLL(so_path)
            if hasattr(lib, "axon_start_nrt_profile"):
                lib.axon_start_nrt_profile.argtypes = [
                    ctypes.POINTER(ctypes.c_int64), ctypes.c_size_t]
                lib.axon_start_nrt_profile.restype = ctypes.c_int64
                lib.axon_stop_nrt_profile.argtypes = [ctypes.c_char_p]
                lib.axon_stop_nrt_profile.restype = ctypes.c_int64

                @contextlib.contextmanager
                def _hook(output_dir, device_ids):
                    import jax
                    jax.devices()
                    if device_ids:
                        ids = (ctypes.c_int64 * len(device_ids))(*device_ids)
                        rc = lib.axon_start_nrt_profile(ids, len(device_ids))
                    else:
                        rc = lib.axon_start_nrt_profile(None, 0)
                    if rc != 0:
                        raise RuntimeError(f"axon_start_nrt_profile rc={rc}")
                    try:
                        yield
                    finally:
                        n = lib.axon_stop_nrt_profile(str(output_dir).encode())
                        if n <= 0:
                            print(f"ntff profile capture wrote {n} files",
                                  file=sys.stderr)

                hook = _hook
        except OSError:
            hook = None

    mod = types.ModuleType("antenv.axon_hooks")
    mod._hook = hook
    mod.get_axon_ntff_profile_hook = lambda: mod._hook

    def _set(h):
        mod._hook = h

    mod.set_axon_ntff_profile_hook = _set
    sys.modules["antenv.axon_hooks"] = mod
    try:
        import antenv
        antenv.axon_hooks = mod
    except ImportError:
        pass


_ensure_ntff_hook()


def _build_nc():
    nc = bacc.Bacc("TRN2", target_bir_lowering=False, debug=False,
                   num_devices=N_CORES)

    x0_d = nc.dram_tensor("x0", [KC, P, 1], F32, kind="ExternalInput")
    w1t_d = nc.dram_tensor("w1t", [KC, P, HID], F32, kind="ExternalInput")
    b1_d = nc.dram_tensor("b1", [HID, 1], F32, kind="ExternalInput")
    w2t_d = nc.dram_tensor("w2t", [HID, 1], F32, kind="ExternalInput")
    b2_d = nc.dram_tensor("b2", [1, 1], F32, kind="ExternalInput")
    xt_d = nc.dram_tensor("xt", [KC, P, N_SH], BF16, kind="ExternalInput")
    xr_d = nc.dram_tensor("xr", [P, MT, D], F32, kind="ExternalInput")
    yt_d = nc.dram_tensor("yt", [NP, P, KC, YP], BF16, kind="ExternalInput")
    out_d = nc.dram_tensor("out", [N_SH, M], F32, kind="ExternalOutput")

    with tile.TileContext(nc) as tc:
        with (
            tc.tile_pool(name="const", bufs=1) as const,
            tc.tile_pool(name="work", bufs=3) as work,
            tc.tile_pool(name="stage", bufs=3) as stage_pool,
            tc.tile_pool(name="ps", bufs=2, space="PSUM") as ps_pool,
        ):
            # ---------------- input DMAs (small/urgent first) -------------
            x0_sb = const.tile([P, KC, 1], F32)
            w1t_sb = const.tile([P, KC, HID], F32)
            b1_sb = const.tile([HID, 1], F32)
            w2t_sb = const.tile([HID, 1], F32)
            b2_sb = const.tile([1, 1], F32)
            for k in range(KC):
                nc.sync.dma_start(x0_sb[:, k], x0_d[k])
                nc.sync.dma_start(w1t_sb[:, k], w1t_d[k])
            nc.sync.dma_start(b1_sb[:], b1_d[:])
            nc.sync.dma_start(w2t_sb[:], w2t_d[:])
            nc.sync.dma_start(b2_sb[:], b2_d[:])

            xT_sb = const.tile([P, KC, N_SH], BF16)
            for k in range(KC):
                nc.sync.dma_start(xT_sb[:, k], xt_d[k])
            xr_sb = const.tile([P, MT, D], F32)
            nc.sync.dma_start(xr_sb[:], xr_d[:])
            yT_sb = const.tile([P, NP, KC, YP], BF16)
            for c in range(NP):
                nc.sync.dma_start(yT_sb[:, c], yt_d[c])

            # ---------------- gamma chain ----------------
            # Funnel the gamma-chain matmul inputs through DVE copies: fp32
            # self-loading matmuls encode only ONE sync wait, but these DMAs
            # land on several DMA semaphore lanes.  After the copies every
            # gamma matmul waits on the single DVE semaphore.
            x0_c = const.tile([P, KC, 1], F32)
            w1t_c = const.tile([P, KC, HID], F32)
            w2t_c = const.tile([HID, 1], F32)
            nc.vector.tensor_copy(x0_c[:], x0_sb[:])
            nc.vector.tensor_copy(w1t_c[:], w1t_sb[:])
            nc.vector.tensor_copy(w2t_c[:], w2t_sb[:])

            ps_h = ps_pool.tile([HID, 1], F32, tag="mm")
            for k in range(KC):
                nc.tensor.matmul(ps_h[:], w1t_c[:, k], x0_c[:, k],
                                 start=(k == 0), stop=(k == KC - 1))
            h_sb = const.tile([HID, 1], F32)
            nc.scalar.activation(h_sb[:], ps_h[:], AF.Relu, bias=b1_sb[:])

            ps_z = ps_pool.tile([1, 1], F32, tag="mm")
            nc.tensor.matmul(ps_z[:], w2t_c[:], h_sb[:], start=True, stop=True)
            u_sb = const.tile([1, 1], F32)
            nc.scalar.activation(u_sb[:], ps_z[:], AF.Exp, bias=b2_sb[:])
            s_sb = const.tile([1, 1], F32)  # softplus(z) = ln(1 + e^z)
            nc.scalar.activation(s_sb[:], u_sb[:], AF.Ln, bias=1.0)

            ones_row = const.tile([1, P], F32)
            nc.vector.memset(ones_row[:], 1.0)
            ps_g = ps_pool.tile([P, 1], F32, tag="mm")
            nc.tensor.matmul(ps_g[:], ones_row[:], s_sb[:], start=True, stop=True)

            negg_f = const.tile([P, 1], F32)     # -gamma on every partition
            nc.vector.tensor_scalar(negg_f[:], ps_g[:], -1.0, -1e-6,
                                    ALU.mult, ALU.add)
            p2g_f = const.tile([P, 1], F32)      # +2*gamma
            nc.vector.tensor_scalar(p2g_f[:], ps_g[:], 2.0, 2e-6,
                                    ALU.mult, ALU.add)

            # ---------------- x-row norms -> per-partition bias -----------
            xn = const.tile([P, MT], F32)
            for m in range(MT):
                sq_scr = work.tile([P, D], F32, tag="sqx")
                nc.vector.scalar_tensor_tensor(
                    sq_scr[:], xr_sb[:, m], 1.0, xr_sb[:, m],
                    ALU.mult, ALU.mult, accum_out=xn[:, m:m + 1])
            negxn = const.tile([P, MT], F32)     # -gamma * ||x_i||^2
            nc.vector.tensor_scalar(negxn[:], xn[:], negg_f[:], None, ALU.mult)

            # constant stationary matrix for the ||y||^2 reduction
            negh = const.tile([P, P], BF16)
            nc.vector.memset(negh[:], -0.5)
            Eyb = const.tile([P, NPAN, PANEL], BF16)

            # ---------------- panels: Ey prep + main loop -----------------
            for p in range(NPAN):
                # Ey[j] = exp(-gamma*||y_j||^2) for this panel's columns
                for c in range(PANEL // YP):
                    piece = p * (PANEL // YP) + c
                    sqy = work.tile([P, KC, YP], BF16, tag="sqy")
                    nc.vector.tensor_tensor(sqy[:], yT_sb[:, piece],
                                            yT_sb[:, piece], ALU.mult)
                    ps_b = ps_pool.tile([P, PANEL], F32, tag="mm")
                    for k in range(KC):
                        for j in range(YP // 512):
                            nc.tensor.matmul(
                                ps_b[:, j * 512:(j + 1) * 512], negh[:],
                                sqy[:, k, j * 512:(j + 1) * 512],
                                start=(k == 0), stop=(k == KC - 1))
                    nc.scalar.activation(
                        Eyb[:, p, c * YP:(c + 1) * YP], ps_b[:, :YP], AF.Exp,
                        scale=p2g_f[:])

                for m in range(MT):
                    msl = slice(m * P, (m + 1) * P)
                    ps = ps_pool.tile([P, PANEL], F32, tag="mm")
                    for k in range(KC):
                        lhsT = xT_sb[:, k, msl]
                        for c in range(PANEL // YP):
                            piece = p * (PANEL // YP) + c
                            for j in range(YP // 512):
                                nc.tensor.matmul(
                                    ps[:, c * YP + j * 512:c * YP + (j + 1) * 512],
                                    lhsT,
                                    yT_sb[:, piece, k, j * 512:(j + 1) * 512],
                                    start=(k == 0), stop=(k == KC - 1))
                    st_in = stage_pool.tile([P, PANEL], BF16, tag="stin")
                    nc.scalar.activation(st_in[:], ps[:], AF.Exp,
                                         bias=negxn[:, m:m + 1], scale=p2g_f[:])
                    st_out = stage_pool.tile([P, PANEL], BF16, tag="stout")
                    nc.vector.tensor_tensor(st_out[:], st_in[:], Eyb[:, p],
                                            ALU.mult)
                    nc.gpsimd.dma_start(
                        out_d[msl, p * PANEL:(p + 1) * PANEL], st_out[:])
    nc.compile()
    return nc


def _get_nc():
    global _NC
    if _NC is None:
        _NC = _build_nc()
    return _NC


def kernel(x, y, W1, b1, W2, b2):
    global LAST_RESULT
    x = np.asarray(x, dtype=np.float32)
    y = np.asarray(y, dtype=np.float32)
    bf = ml_dtypes.bfloat16

    # y^T piece-major [NP, 128, KC, YP]: [c, p, k, j] = y[c*YP+j, k*128+p]
    yt = np.ascontiguousarray(
        y.T.reshape(KC, P, NP, YP).transpose(2, 1, 0, 3)).astype(bf)
    x0 = np.ascontiguousarray(x[0]).reshape(KC, P, 1).astype(np.float32)
    w1t = np.ascontiguousarray(np.asarray(W1, np.float32).T).reshape(KC, P, HID)
    b1c = np.asarray(b1, np.float32).reshape(HID, 1)
    w2t = np.ascontiguousarray(np.asarray(W2, np.float32).T).reshape(HID, 1)
    b2c = np.asarray(b2, np.float32).reshape(1, 1)

    in_maps = []
    for c in range(N_CORES):
        shard = x[c * N_SH:(c + 1) * N_SH]
        xt = np.ascontiguousarray(shard.T).reshape(KC, P, N_SH).astype(bf)
        xr = np.ascontiguousarray(
            shard.reshape(MT, P, D).transpose(1, 0, 2))
        in_maps.append({"xt": xt, "xr": xr, "yt": yt, "x0": x0, "w1t": w1t,
                        "b1": b1c, "w2t": w2t, "b2": b2c})

    nc = _get_nc()
    LAST_RESULT = run_bass_kernel_spmd(nc, in_maps, core_ids=list(range(N_CORES)))
    return np.concatenate([LAST_RESULT.results[c]["out"]
                           for c in range(N_CORES)], axis=0)


# revision 4
# speedup vs baseline: 1.9374x; 1.2574x over previous
"""RBF kernel matrix on 8 TRN2 NeuronCores.

Computes out[i, j] = exp(-gamma * (||x_i||^2 + ||y_j||^2 - 2 x_i.y_j))
with gamma = softplus(MLP(x[0])) + 1e-6, as a Bass/Tile SPMD kernel.

Sharding: rows of x across the 8 cores (1024 rows each); y and the tiny
gamma-net parameters are replicated.  Each core computes its (1024, 8192)
slab of the output; the host concatenates the slabs and widens bf16->f32
(the on-device pipeline is bf16 end-to-end after the exp, so the widening
is exact -- same numbers, half the HBM write traffic).

Per-core device pipeline (keeps the PE free of norm-handling matmuls):
  - gamma chain (TensorE f32 matmuls + ACT relu / exp / ln -> softplus)
    from one packed 19 KB parameter DMA
  - Ey[j] = exp(-gamma*||y_j||^2) built once per panel: DVE squares y^T,
    K=128 matmuls against a constant (-0.5) stationary matrix reduce
    them, ACT exponentiates with the per-partition 2*gamma scale
  - x-row norms from native x via DVE square+accum -> per-partition bias
  - main loop per [128, 2048] tile: psum = x.y (bf16 K=128 matmuls), ACT
    stage = exp(2g*psum - g*||x_i||^2) in bf16, DVE stage *= Ey (bf16 2x
    mode), HWDGE DMA of the bf16 tile to DRAM.
The multiplicative split exp(a+b) = exp(a)*exp(b) is safe here: both
factors are ~exp(-gamma*O(d)) << 1 for this input distribution, so
neither factor can overflow on its own.
"""

import numpy as np
import ml_dtypes

import concourse.bacc as bacc
import concourse.bass as bass  # noqa: F401
import concourse.mybir as mybir
import concourse.tile as tile
from concourse.bass_utils import run_bass_kernel_spmd

N_CORES = 8
N, M, D = 8192, 8192, 256
N_SH = N // N_CORES  # rows of x per core
HID = 16
P = 128
KC = D // P  # k-chunks (2)
MT = N_SH // P  # m-tiles per core (8)
YP = 1024  # y columns per input-DMA piece
NP = M // YP  # pieces (8)
PANEL = 2048  # main-loop columns per panel / psum tile
NPAN = M // PANEL  # panels (4)
GP_COLS = 37  # packed gamma-net params: w1t|w1t|x0|x0|b1|w2t|b2

F32 = mybir.dt.float32
BF16 = mybir.dt.bfloat16
AF = mybir.ActivationFunctionType
ALU = mybir.AluOpType

_NC = None
LAST_RESULT = None


def _ensure_ntff_hook():
    """Register an ``antenv.axon_hooks`` shim if the image lacks it.

    ``run_bass_kernel_spmd(trace=True)`` under axon imports
    ``antenv.axon_hooks.get_axon_ntff_profile_hook``; some images miss the
    module, which would crash tracing.  Recreate the boot-script hook via
    ctypes against libaxon_pjrt.so, degrading to hook=None when absent.
    """
    import contextlib
    import ctypes
    import os
    import sys
    import types

    try:
        import antenv.axon_hooks  # noqa: F401
        return
    except ImportError:
        pass

    hook = None
    so_path = "/opt/axon/libaxon_pjrt.so"
    if os.path.exists(so_path):
        try:
            lib = ctypes.CDLL(so_path)
            if hasattr(lib, "axon_start_nrt_profile"):
                lib.axon_start_nrt_profile.argtypes = [
                    ctypes.POINTER(ctypes.c_int64), ctypes.c_size_t]
                lib.axon_start_nrt_profile.restype = ctypes.c_int64
                lib.axon_stop_nrt_profile.argtypes = [ctypes.c_char_p]
                lib.axon_stop_nrt_profile.restype = ctypes.c_int64

                @contextlib.contextmanager
                def _hook(output_dir, device_ids):
                    import jax
                    jax.devices()
                    if device_ids:
                        ids = (ctypes.c_int64 * len(device_ids))(*device_ids)
                        rc = lib.axon_start_nrt_profile(ids, len(device_ids))
                    else:
                        rc = lib.axon_start_nrt_profile(None, 0)
                    if rc != 0:
                        raise RuntimeError(f"axon_start_nrt_profile rc={rc}")
                    try:
                        yield
                    finally:
                        n = lib.axon_stop_nrt_profile(str(output_dir).encode())
                        if n <= 0:
                            print(f"ntff profile capture wrote {n} files",
                                  file=sys.stderr)

                hook = _hook
        except OSError:
            hook = None

    mod = types.ModuleType("antenv.axon_hooks")
    mod._hook = hook
    mod.get_axon_ntff_profile_hook = lambda: mod._hook

    def _set(h):
        mod._hook = h

    mod.set_axon_ntff_profile_hook = _set
    sys.modules["antenv.axon_hooks"] = mod
    try:
        import antenv
        antenv.axon_hooks = mod
    except ImportError:
        pass


_ensure_ntff_hook()


def _build_nc():
    nc = bacc.Bacc("TRN2", target_bir_lowering=False, debug=False,
                   num_devices=N_CORES)

    gp_d = nc.dram_tensor("gp", [P, GP_COLS], F32, kind="ExternalInput")
    xt_d = nc.dram_tensor("xt", [P, KC, N_SH], BF16, kind="ExternalInput")
    xr_d = nc.dram_tensor("xr", [P, MT, D], F32, kind="ExternalInput")
    yt_d = nc.dram_tensor("yt", [NP, P, KC, YP], BF16, kind="ExternalInput")
    out_d = nc.dram_tensor("out", [N_SH, M], BF16, kind="ExternalOutput")

    with tile.TileContext(nc) as tc:
        with (
            tc.tile_pool(name="const", bufs=1) as const,
            tc.tile_pool(name="work", bufs=3) as work,
            tc.tile_pool(name="stage", bufs=3) as stage_pool,
            tc.tile_pool(name="ps", bufs=2, space="PSUM") as ps_pool,
        ):
            # ------------- input DMAs (gamma/x on the ACT HWDGE ring, -----
            # ------------- y pieces on the Sync ring) ---------------------
            gp = const.tile([P, GP_COLS], F32)
            nc.scalar.dma_start(gp[:], gp_d[:])
            xT_sb = const.tile([P, KC, N_SH], BF16)
            nc.scalar.dma_start(xT_sb[:], xt_d[:])
            xr_sb = const.tile([P, MT, D], F32)
            nc.scalar.dma_start(xr_sb[:], xr_d[:])
            yT_sb = const.tile([P, NP, KC, YP], BF16)
            for c in range(NP):
                nc.sync.dma_start(yT_sb[:, c], yt_d[c])

            # ---------------- gamma chain ----------------
            ps_h = ps_pool.tile([HID, 1], F32, tag="mm")
            for k in range(KC):
                nc.tensor.matmul(ps_h[:], gp[:, k * HID:(k + 1) * HID],
                                 gp[:, 32 + k:33 + k],
                                 start=(k == 0), stop=(k == KC - 1))
            h_sb = const.tile([HID, 1], F32)
            nc.scalar.activation(h_sb[:], ps_h[:], AF.Relu,
                                 bias=gp[0:HID, 34:35])

            ps_z = ps_pool.tile([1, 1], F32, tag="mm")
            nc.tensor.matmul(ps_z[:], gp[0:HID, 35:36], h_sb[:],
                             start=True, stop=True)
            u_sb = const.tile([1, 1], F32)
            nc.scalar.activation(u_sb[:], ps_z[:], AF.Exp, bias=gp[0:1, 36:37])
            s_sb = const.tile([1, 1], F32)  # softplus(z) = ln(1 + e^z)
            nc.scalar.activation(s_sb[:], u_sb[:], AF.Ln, bias=1.0)

            ones_row = const.tile([1, P], F32)
            nc.vector.memset(ones_row[:], 1.0)
            ps_g = ps_pool.tile([P, 1], F32, tag="mm")
            nc.tensor.matmul(ps_g[:], ones_row[:], s_sb[:], start=True, stop=True)

            negg_f = const.tile([P, 1], F32)     # -gamma on every partition
            nc.vector.tensor_scalar(negg_f[:], ps_g[:], -1.0, -1e-6,
                                    ALU.mult, ALU.add)
            p2g_f = const.tile([P, 1], F32)      # +2*gamma
            nc.vector.tensor_scalar(p2g_f[:], ps_g[:], 2.0, 2e-6,
                                    ALU.mult, ALU.add)

            # ---------------- x-row norms -> per-partition bias -----------
            xn = const.tile([P, MT], F32)
            for m in range(MT):
                sq_scr = work.tile([P, D], F32, tag="sqx")
                nc.vector.scalar_tensor_tensor(
                    sq_scr[:], xr_sb[:, m], 1.0, xr_sb[:, m],
                    ALU.mult, ALU.mult, accum_out=xn[:, m:m + 1])
            negxn = const.tile([P, MT], F32)     # -gamma * ||x_i||^2
            nc.vector.tensor_scalar(negxn[:], xn[:], negg_f[:], None, ALU.mult)

            # constant stationary matrix for the ||y||^2 reduction
            negh = const.tile([P, P], BF16)
            nc.vector.memset(negh[:], -0.5)
            Eyb = const.tile([P, NPAN, PANEL], BF16)

            # ---------------- panels: Ey prep + main loop -----------------
            for p in range(NPAN):
                # Ey[j] = exp(-gamma*||y_j||^2) for this panel's columns
                for c in range(PANEL // YP):
                    piece = p * (PANEL // YP) + c
                    sqy = work.tile([P, KC, YP], BF16, tag="sqy")
                    nc.vector.tensor_tensor(sqy[:], yT_sb[:, piece],
                                            yT_sb[:, piece], ALU.mult)
                    ps_b = ps_pool.tile([P, PANEL], F32, tag="mm")
                    for k in range(KC):
                        for j in range(YP // 512):
                            nc.tensor.matmul(
                                ps_b[:, j * 512:(j + 1) * 512], negh[:],
                                sqy[:, k, j * 512:(j + 1) * 512],
                                start=(k == 0), stop=(k == KC - 1))
                    nc.scalar.activation(
                        Eyb[:, p, c * YP:(c + 1) * YP], ps_b[:, :YP], AF.Exp,
                        scale=p2g_f[:])

                for m in range(MT):
                    msl = slice(m * P, (m + 1) * P)
                    ps = ps_pool.tile([P, PANEL], F32, tag="mm")
                    for k in range(KC):
                        lhsT = xT_sb[:, k, msl]
                        for c in range(PANEL // YP):
                            piece = p * (PANEL // YP) + c
                            for j in range(YP // 512):
                                nc.tensor.matmul(
                                    ps[:, c * YP + j * 512:c * YP + (j + 1) * 512],
                                    lhsT,
                                    yT_sb[:, piece, k, j * 512:(j + 1) * 512],
                                    start=(k == 0), stop=(k == KC - 1))
                    st_in = stage_pool.tile([P, PANEL], BF16, tag="stin")
                    nc.scalar.activation(st_in[:], ps[:], AF.Exp,
                                         bias=negxn[:, m:m + 1], scale=p2g_f[:])
                    st_out = stage_pool.tile([P, PANEL], BF16, tag="stout")
                    nc.vector.tensor_tensor(st_out[:], st_in[:], Eyb[:, p],
                                            ALU.mult)
                    nc.sync.dma_start(
                        out_d[msl, p * PANEL:(p + 1) * PANEL], st_out[:])
    nc.compile()
    return nc


def _get_nc():
    global _NC
    if _NC is None:
        _NC = _build_nc()
    return _NC


def kernel(x, y, W1, b1, W2, b2):
    global LAST_RESULT
    x = np.asarray(x, dtype=np.float32)
    y = np.asarray(y, dtype=np.float32)
    bf = ml_dtypes.bfloat16

    # y^T piece-major [NP, 128, KC, YP]: [c, p, k, j] = y[c*YP+j, k*128+p]
    yt = np.ascontiguousarray(
        y.T.reshape(KC, P, NP, YP).transpose(2, 1, 0, 3)).astype(bf)

    gp = np.zeros((P, GP_COLS), np.float32)
    w1T = np.asarray(W1, np.float32).T  # (D, HID)
    gp[:, 0:HID] = w1T[0:P]
    gp[:, HID:2 * HID] = w1T[P:2 * P]
    gp[:, 32] = x[0, 0:P]
    gp[:, 33] = x[0, P:2 * P]
    gp[0:HID, 34] = np.asarray(b1, np.float32)
    gp[0:HID, 35] = np.asarray(W2, np.float32).reshape(HID)
    gp[0, 36] = np.asarray(b2, np.float32).reshape(1)[0]

    in_maps = []
    for c in range(N_CORES):
        shard = x[c * N_SH:(c + 1) * N_SH]
        xt = np.ascontiguousarray(
            shard.T.reshape(KC, P, N_SH).transpose(1, 0, 2)).astype(bf)
        xr = np.ascontiguousarray(
            shard.reshape(MT, P, D).transpose(1, 0, 2))
        in_maps.append({"gp": gp, "xt": xt, "xr": xr, "yt": yt})

    nc = _get_nc()
    LAST_RESULT = run_bass_kernel_spmd(nc, in_maps, core_ids=list(range(N_CORES)))
    out = np.empty((N, M), np.float32)
    for c in range(N_CORES):
        out[c * N_SH:(c + 1) * N_SH] = LAST_RESULT.results[c]["out"]
    return out


# revision 6
# speedup vs baseline: 2.0949x; 1.0813x over previous
"""RBF kernel matrix on 8 TRN2 NeuronCores.

Computes out[i, j] = exp(-gamma * (||x_i||^2 + ||y_j||^2 - 2 x_i.y_j))
with gamma = softplus(MLP(x[0])) + 1e-6, as a Bass/Tile SPMD kernel.

Sharding: rows of x across the 8 cores (1024 rows each); y and the tiny
gamma-net parameters are replicated.  Each core computes its (1024, 8192)
slab of the output; the host concatenates the slabs and widens bf16->f32
(the on-device pipeline is bf16 end-to-end after the exp, so the widening
is exact -- same numbers, half the HBM write traffic).

Per-core device pipeline (keeps the PE free of norm-handling matmuls):
  - gamma chain (TensorE f32 matmuls + ACT relu / exp / ln -> softplus)
    from one packed 19 KB parameter DMA
  - Ey[j] = exp(-gamma*||y_j||^2) built once per panel: DVE squares y^T,
    K=128 matmuls against a constant (-0.5) stationary matrix reduce
    them, ACT exponentiates with the per-partition 2*gamma scale
  - x-row norms from native x via DVE square+accum -> per-partition bias
  - main loop per [128, 2048] tile: psum = x.y (bf16 K=128 matmuls), ACT
    stage = exp(2g*psum - g*||x_i||^2) in bf16, DVE stage *= Ey (bf16 2x
    mode), HWDGE DMA of the bf16 tile to DRAM.
The multiplicative split exp(a+b) = exp(a)*exp(b) is safe here: both
factors are ~exp(-gamma*O(d)) << 1 for this input distribution, so
neither factor can overflow on its own.
"""

import numpy as np
import ml_dtypes

import concourse.bacc as bacc
import concourse.bass as bass  # noqa: F401
import concourse.mybir as mybir
import concourse.tile as tile
from concourse.bass_utils import run_bass_kernel_spmd

N_CORES = 8
N, M, D = 8192, 8192, 256
N_SH = N // N_CORES  # rows of x per core
HID = 16
P = 128
KC = D // P  # k-chunks (2)
MT = N_SH // P  # m-tiles per core (8)
YP = 1024  # y columns per input-DMA piece
NP = M // YP  # pieces (8)
PANEL = 2048  # main-loop columns per panel / psum tile
NPAN = M // PANEL  # panels (4)
GP_COLS = 37  # packed gamma-net params: w1t|w1t|x0|x0|b1|w2t|b2

F32 = mybir.dt.float32
BF16 = mybir.dt.bfloat16
AF = mybir.ActivationFunctionType
ALU = mybir.AluOpType

_NC = None
LAST_RESULT = None


def _ensure_ntff_hook():
    """Register an ``antenv.axon_hooks`` shim if the image lacks it.

    ``run_bass_kernel_spmd(trace=True)`` under axon imports
    ``antenv.axon_hooks.get_axon_ntff_profile_hook``; some images miss the
    module, which would crash tracing.  Recreate the boot-script hook via
    ctypes against libaxon_pjrt.so, degrading to hook=None when absent.
    """
    import contextlib
    import ctypes
    import os
    import sys
    import types

    try:
        import antenv.axon_hooks  # noqa: F401
        return
    except ImportError:
        pass

    hook = None
    so_path = "/opt/axon/libaxon_pjrt.so"
    if os.path.exists(so_path):
        try:
            lib = ctypes.CDLL(so_path)
            if hasattr(lib, "axon_start_nrt_profile"):
                lib.axon_start_nrt_profile.argtypes = [
                    ctypes.POINTER(ctypes.c_int64), ctypes.c_size_t]
                lib.axon_start_nrt_profile.restype = ctypes.c_int64
                lib.axon_stop_nrt_profile.argtypes = [ctypes.c_char_p]
                lib.axon_stop_nrt_profile.restype = ctypes.c_int64

                @contextlib.contextmanager
                def _hook(output_dir, device_ids):
                    import jax
                    jax.devices()
                    if device_ids:
                        ids = (ctypes.c_int64 * len(device_ids))(*device_ids)
                        rc = lib.axon_start_nrt_profile(ids, len(device_ids))
                    else:
                        rc = lib.axon_start_nrt_profile(None, 0)
                    if rc != 0:
                        raise RuntimeError(f"axon_start_nrt_profile rc={rc}")
                    try:
                        yield
                    finally:
                        n = lib.axon_stop_nrt_profile(str(output_dir).encode())
                        if n <= 0:
                            print(f"ntff profile capture wrote {n} files",
                                  file=sys.stderr)

                hook = _hook
        except OSError:
            hook = None

    mod = types.ModuleType("antenv.axon_hooks")
    mod._hook = hook
    mod.get_axon_ntff_profile_hook = lambda: mod._hook

    def _set(h):
        mod._hook = h

    mod.set_axon_ntff_profile_hook = _set
    sys.modules["antenv.axon_hooks"] = mod
    try:
        import antenv
        antenv.axon_hooks = mod
    except ImportError:
        pass


_ensure_ntff_hook()


def _build_nc():
    nc = bacc.Bacc("TRN2", target_bir_lowering=False, debug=False,
                   num_devices=N_CORES)

    gp_d = nc.dram_tensor("gp", [P, GP_COLS], F32, kind="ExternalInput")
    xt_d = nc.dram_tensor("xt", [P, KC, N_SH], BF16, kind="ExternalInput")
    xr_d = nc.dram_tensor("xr", [P, MT, D], F32, kind="ExternalInput")
    yt_d = nc.dram_tensor("yt", [NP, P, KC, YP], BF16, kind="ExternalInput")
    out_d = nc.dram_tensor("out", [N_SH, M], BF16, kind="ExternalOutput")

    with tile.TileContext(nc) as tc:
        with (
            tc.tile_pool(name="const", bufs=1) as const,
            tc.tile_pool(name="work", bufs=3) as work,
            tc.tile_pool(name="stage", bufs=3) as stage_pool,
            tc.tile_pool(name="ps", bufs=2, space="PSUM") as ps_pool,
        ):
            # ------------- input DMAs (gamma/y on the Sync HWDGE ring, ----
            # ------------- x tensors on the ACT ring) ---------------------
            gp = const.tile([P, GP_COLS], F32)
            nc.sync.dma_start(gp[:], gp_d[:])
            yT_sb = const.tile([P, NP, KC, YP], BF16)
            for c in range(NP):
                nc.sync.dma_start(yT_sb[:, c], yt_d[c])
            xT_sb = const.tile([P, KC, N_SH], BF16)
            nc.scalar.dma_start(xT_sb[:], xt_d[:])
            xr_sb = const.tile([P, MT, D], F32)
            nc.scalar.dma_start(xr_sb[:], xr_d[:])

            # DVE constants first: no dependencies, run during the boot gap
            ones_row = const.tile([1, P], F32)
            nc.vector.memset(ones_row[:], 1.0)
            negh = const.tile([P, P], BF16)  # stationary -0.5 for ||y||^2
            nc.vector.memset(negh[:], -0.5)

            # ---------------- gamma chain ----------------
            ps_h = ps_pool.tile([HID, 1], F32, tag="mm")
            for k in range(KC):
                nc.tensor.matmul(ps_h[:], gp[:, k * HID:(k + 1) * HID],
                                 gp[:, 32 + k:33 + k],
                                 start=(k == 0), stop=(k == KC - 1))
            h_sb = const.tile([HID, 1], F32)  # relu(W1 x0 + b1) on the DVE
            nc.vector.tensor_scalar(h_sb[:], ps_h[:], gp[0:HID, 34:35], 0.0,
                                    ALU.add, ALU.max)

            ps_z = ps_pool.tile([1, 1], F32, tag="mm")
            nc.tensor.matmul(ps_z[:], gp[0:HID, 35:36], h_sb[:],
                             start=True, stop=True)
            u_sb = const.tile([1, 1], F32)
            nc.scalar.activation(u_sb[:], ps_z[:], AF.Exp, bias=gp[0:1, 36:37])
            s_sb = const.tile([1, 1], F32)  # softplus(z) = ln(1 + e^z)
            nc.scalar.activation(s_sb[:], u_sb[:], AF.Ln, bias=1.0)

            ps_g = ps_pool.tile([P, 1], F32, tag="mm")
            nc.tensor.matmul(ps_g[:], ones_row[:], s_sb[:], start=True, stop=True)

            negg_f = const.tile([P, 1], F32)     # -gamma on every partition
            nc.vector.tensor_scalar(negg_f[:], ps_g[:], -1.0, -1e-6,
                                    ALU.mult, ALU.add)
            p2g_f = const.tile([P, 1], F32)      # +2*gamma
            nc.vector.tensor_scalar(p2g_f[:], ps_g[:], 2.0, 2e-6,
                                    ALU.mult, ALU.add)

            xn = const.tile([P, MT], F32)
            negxn = const.tile([P, MT], F32)     # -gamma * ||x_i||^2
            Eyb = const.tile([P, NPAN, PANEL], BF16)

            # ---------------- panels: Ey prep + main loop -----------------
            for p in range(NPAN):
                # Ey[j] = exp(-gamma*||y_j||^2) for this panel's columns
                ps_b = ps_pool.tile([P, PANEL], F32, tag="mm")
                for c in range(PANEL // YP):
                    piece = p * (PANEL // YP) + c
                    sqy = work.tile([P, KC, YP], BF16, tag="sqy")
                    nc.vector.tensor_tensor(sqy[:], yT_sb[:, piece],
                                            yT_sb[:, piece], ALU.mult)
                    for k in range(KC):
                        for j in range(YP // 512):
                            nc.tensor.matmul(
                                ps_b[:, c * YP + j * 512:c * YP + (j + 1) * 512],
                                negh[:], sqy[:, k, j * 512:(j + 1) * 512],
                                start=(k == 0), stop=(k == KC - 1))
                nc.scalar.activation(Eyb[:, p], ps_b[:], AF.Exp,
                                     scale=p2g_f[:])

                if p == 0:
                    # x-row norms -> per-partition bias (needed by the first
                    # main-loop exp; emitted after panel-0 sqy so the DVE
                    # queue never stalls on the slower xr DMA)
                    for m in range(MT):
                        sq_scr = work.tile([P, D], F32, tag="sqx")
                        nc.vector.scalar_tensor_tensor(
                            sq_scr[:], xr_sb[:, m], 1.0, xr_sb[:, m],
                            ALU.mult, ALU.mult, accum_out=xn[:, m:m + 1])
                    nc.vector.tensor_scalar(negxn[:], xn[:], negg_f[:], None,
                                            ALU.mult)

                for m in range(MT):
                    msl = slice(m * P, (m + 1) * P)
                    ps = ps_pool.tile([P, PANEL], F32, tag="mm")
                    for k in range(KC):
                        lhsT = xT_sb[:, k, msl]
                        for c in range(PANEL // YP):
                            piece = p * (PANEL // YP) + c
                            for j in range(YP // 512):
                                nc.tensor.matmul(
                                    ps[:, c * YP + j * 512:c * YP + (j + 1) * 512],
                                    lhsT,
                                    yT_sb[:, piece, k, j * 512:(j + 1) * 512],
                                    start=(k == 0), stop=(k == KC - 1))
                    st_in = stage_pool.tile([P, PANEL], BF16, tag="stin")
                    nc.scalar.activation(st_in[:], ps[:], AF.Exp,
                                         bias=negxn[:, m:m + 1], scale=p2g_f[:])
                    st_out = stage_pool.tile([P, PANEL], BF16, tag="stout")
                    nc.vector.tensor_tensor(st_out[:], st_in[:], Eyb[:, p],
                                            ALU.mult)
                    nc.sync.dma_start(
                        out_d[msl, p * PANEL:(p + 1) * PANEL], st_out[:])
    nc.compile()
    return nc


def _get_nc():
    global _NC
    if _NC is None:
        _NC = _build_nc()
    return _NC


def kernel(x, y, W1, b1, W2, b2):
    global LAST_RESULT
    x = np.asarray(x, dtype=np.float32)
    y = np.asarray(y, dtype=np.float32)
    bf = ml_dtypes.bfloat16

    # y^T piece-major [NP, 128, KC, YP]: [c, p, k, j] = y[c*YP+j, k*128+p]
    yt = np.ascontiguousarray(
        y.T.reshape(KC, P, NP, YP).transpose(2, 1, 0, 3)).astype(bf)

    gp = np.zeros((P, GP_COLS), np.float32)
    w1T = np.asarray(W1, np.float32).T  # (D, HID)
    gp[:, 0:HID] = w1T[0:P]
    gp[:, HID:2 * HID] = w1T[P:2 * P]
    gp[:, 32] = x[0, 0:P]
    gp[:, 33] = x[0, P:2 * P]
    gp[0:HID, 34] = np.asarray(b1, np.float32)
    gp[0:HID, 35] = np.asarray(W2, np.float32).reshape(HID)
    gp[0, 36] = np.asarray(b2, np.float32).reshape(1)[0]

    in_maps = []
    for c in range(N_CORES):
        shard = x[c * N_SH:(c + 1) * N_SH]
        xt = np.ascontiguousarray(
            shard.T.reshape(KC, P, N_SH).transpose(1, 0, 2)).astype(bf)
        xr = np.ascontiguousarray(
            shard.reshape(MT, P, D).transpose(1, 0, 2))
        in_maps.append({"gp": gp, "xt": xt, "xr": xr, "yt": yt})

    nc = _get_nc()
    LAST_RESULT = run_bass_kernel_spmd(nc, in_maps, core_ids=list(range(N_CORES)))
    out = np.empty((N, M), np.float32)
    for c in range(N_CORES):
        out[c * N_SH:(c + 1) * N_SH] = LAST_RESULT.results[c]["out"]
    return out
